# revision 99
# baseline (speedup 1.0000x reference)
"""Multi-head attention (softmax over the QUERY axis) for Trainium2, 8 cores.

Reference (B=2, T=2048, E=1024, H=16, HD=64):
    q = X@Wq.T+bq ; k = ... ; v = ...   (per-head split)
    s = (q k^T)/sqrt(E), causal mask (key > query -> -inf)
    attn = softmax(s, axis=QUERY)  -> normalizes each key COLUMN over queries
    out  = attn @ v

Sharding: core c = (batch c//4, head-group c%4 of 4 heads = 2 duos).  No
collectives.

Math per core (d2=256 output dims, O stored [T, 256]):
  Projections run in fp8e4 DoubleRow with hi+lo error compensation:
  W' = 32*W split W8+Wl (fp8), X split X8+Xl; q32 ~= X8@W8 + Xl@W8 + X8@Wl
  (+32b).  Scores use bf16 Q/K: s_psum = 1024*s; exp applies
  scale=1/32768 and per-key bias -ln(c_k), c_k ~ sqrt(E[r_k]) (any c_k is
  mathematically exact: V-scaling self-corrects).  P~=P/c_k stored fp8
  (keys < 1792) / fp16 (last 256 keys); V~ = 32*v*c/r.  A@V runs fp8
  DoubleRow over PAIRS of key-tiles (contraction 256), P~ stationary and
  V~ moving with both heads side-by-side; within each A@V q-tile the
  pair matmuls AND the kt14/15 fp16 tail matmuls stay inside one per-head
  loop so each psum accumulation group opens and closes with no other
  group starting on the same bank in between (interleaving them resets
  the open accumulator and corrupts q-tiles 14/15).  Final PSUM->SBUF
  copy multiplies by 1/32.

Schedule (ACT = the bottleneck: ~89us busy):
  duo0 DESCENDING key-tiles / duo1 ASCENDING, as one continuous exp
  stream.  Startup: a single small fp8 DMA (cst8h: X cols [1920:2048],
  exp-bias table) + the first wqk chunk land ~4.3us in; ~36 dummy
  matmuls on a zeroed strip burn the PE p-state ramp during the DMA
  window so the first projection chunks run at full clock; the first exp
  fires ~6us in (vs ~10us).  Q/K projection chunks are 128-256 cols wide
  and sit at their LATEST deadline (verified by build-time assertions on
  written-column coverage); each unit's filler PE-time fits its own exp
  window.  Scores piece-2 (queries beyond 1536) borrows a 512-wide
  proj/V psum slot so the two 1536-wide score slots ping-pong among
  full-width pieces only -> no slot-reuse bubbles on big key-tiles.
  duo0's V~ fills and A@V q-tiles inject into duo1's phase at their
  latest-safe units; duo0-kt0's unit carries no leftover fillers so
  duo1-kt0's scores chain straight through the seam.
PSUM: 2x[128,1536] score slots + 2x[128,512] proj/V/A@V/piece2 slots.
"""

import math
from contextlib import ExitStack

import numpy as np
import ml_dtypes

import concourse.bacc as bacc
import concourse.mybir as mybir
import concourse.tile as tile
from concourse.bass_utils import run_bass_kernel_spmd

B, T, E, H = 2, 2048, 1024, 16
D2 = 256              # output dims per core (4 heads)
NKT = 16              # key tiles of 128
NPAIR = 7             # fp8 DoubleRow pairs: kts (2p, 2p+1), p=0..6; kts 14,15 fp16
F32 = mybir.dt.float32
BF16 = mybir.dt.bfloat16
F16 = mybir.dt.float16
F8 = mybir.dt.float8e4
DRM = mybir.MatmulPerfMode.DoubleRow
EXP = mybir.ActivationFunctionType.Exp
AX = mybir.AxisListType.X
SCALE = 1.0 / 32768.0   # 1/sqrt(E) / 32 / 32  (both Q,K carry 32x weights)
NEG = -1.0e30
WS = 32.0               # weight prescale
XMAIN = 1792            # x8/xl main-tile cols [0:1792]; tail packed in cst8
np_f8 = ml_dtypes.float8_e4m3
np_bf16 = ml_dtypes.bfloat16

# w_t column-block offsets (per e-chunk, 1536 wide; Q/K are duo-major so
# the head only needs the first 512 W columns)
V8O, VLO = 1024, 1280

_CACHE = {}


def _build_module():
    nc = bacc.Bacc("TRN2", target_bir_lowering=False, debug=False)

    # cst8h planes (width 128B): 0-7 x8[:, :, 1920:2048], 8-15 xl[...],
    # 16 cf bytes (biasc|bqc|bkc f32 = 80B) -- one small DMA carries the
    # X tail + biases the very first score-exp unit needs.
    cst8h_d = nc.dram_tensor("cst8h", [128, 17, 128], F8,
                             kind="ExternalInput")
    # bf16 consts as one plain 2D tensor (3D fp8 bitcast views are not a
    # safe matmul operand): masku | ident | ones | bvr(row 0)
    mi_d = nc.dram_tensor("mi", [128, 640], BF16, kind="ExternalInput")
    # cst8b planes: 0-7 x8[:, :, 1792:1920], 8-15 xl[...]
    cst8b_d = nc.dram_tensor("cst8b", [128, 16, 128], F8,
                             kind="ExternalInput")
    x8_d = nc.dram_tensor("x8", [128, 8, XMAIN], F8, kind="ExternalInput")
    xl_d = nc.dram_tensor("xl", [128, 8, XMAIN], F8, kind="ExternalInput")
    wqk_d = nc.dram_tensor("wqk", [128, 8, 1024], F8, kind="ExternalInput")
    wv8l_d = nc.dram_tensor("wv8l", [128, 8, 512], F8, kind="ExternalInput")
    ot_d = nc.dram_tensor("ot", [T, D2], F32, kind="ExternalOutput")

    with tile.TileContext(nc) as tc:
        _body(tc, cst8h_d, cst8b_d, mi_d, x8_d, xl_d, wqk_d,
              wv8l_d, ot_d)
    nc.compile()
    return nc


def _body(tc, cst8h_d, cst8b_d, mi_d, x8_d, xl_d, wqk_d, wv8l_d,
          ot_d):
    nc = tc.nc

    with ExitStack() as ctx:
        cp = ctx.enter_context(tc.tile_pool(name="const", bufs=1))
        cst8h_t = cp.tile([128, 17, 128], F8)
        cst8b_t = cp.tile([128, 16, 128], F8)
        mi_t = cp.tile([128, 640], BF16)
        x8h_t = cst8h_t[:, 0:8, :]      # x cols [1920:2048]
        xlh_t = cst8h_t[:, 8:16, :]
        x8b_t = cst8b_t[:, 0:8, :]      # x cols [1792:1920]
        xlb_t = cst8b_t[:, 8:16, :]
        cf_t = cst8h_t[:, 16, 0:80].bitcast(F32)      # [128, 20]
        biasc_t = cf_t[:, 0:NKT]
        bqc_t = cf_t[:, NKT:NKT + 2]
        bkc_t = cf_t[:, NKT + 2:NKT + 4]
        masku_t = mi_t[:, 0:128]
        ident_t = mi_t[:, 128:256]
        ones_t = mi_t[:, 256:384]
        bvr_t = mi_t[0:1, 384:640]                    # [1, 256] row 0

        xw = ctx.enter_context(tc.tile_pool(name="xw", bufs=1))
        x8_t = xw.tile([128, 8, XMAIN], F8)
        xl_t = xw.tile([128, 8, XMAIN], F8)
        w_t = xw.tile([128, 8, 1536], F8)

        qk = ctx.enter_context(tc.tile_pool(name="qk", bufs=1))
        qt_t = qk.tile([128, 2, T], BF16)
        kt_t = qk.tile([128, 2, T], BF16)

        vtp = ctx.enter_context(tc.tile_pool(name="vt", bufs=1))
        v_t = vtp.tile([128, NKT, D2], BF16)

        pp = ctx.enter_context(tc.tile_pool(name="pp", bufs=1))
        vp = ctx.enter_context(tc.tile_pool(name="vp", bufs=1))
        st = ctx.enter_context(tc.tile_pool(name="st", bufs=6))
        osb = ctx.enter_context(tc.tile_pool(name="osb", bufs=1))

        sc_pool = ctx.enter_context(
            tc.tile_pool(name="scp", bufs=2, space="PSUM"))
        pv_pool = ctx.enter_context(
            tc.tile_pool(name="pvp", bufs=1, space="PSUM"))

        # ---- input DMA: the small packed const transfer goes first so
        # the first score-exp unit (duo0 kt15: X cols 1920:2048, mask,
        # bias) depends on just cst8h + the first wqk chunk ----
        nc.sync.dma_start(w_t[:, :, 0:512], wqk_d.ap()[:, :, 0:512])
        nc.sync.dma_start(cst8h_t[:], cst8h_d.ap())
        nc.sync.dma_start(cst8b_t[:], cst8b_d.ap())
        nc.sync.dma_start(mi_t[:], mi_d.ap())
        nc.sync.dma_start(x8_t[:, :, 1280:1792], x8_d.ap()[:, :, 1280:1792])
        nc.sync.dma_start(xl_t[:, :, 1280:1792], xl_d.ap()[:, :, 1280:1792])
        nc.sync.dma_start(x8_t[:, :, 768:1280], x8_d.ap()[:, :, 768:1280])
        nc.sync.dma_start(xl_t[:, :, 768:1280], xl_d.ap()[:, :, 768:1280])
        nc.sync.dma_start(w_t[:, :, 1024:1536], wv8l_d.ap())
        nc.sync.dma_start(w_t[:, :, 512:1024], wqk_d.ap()[:, :, 512:1024])
        nc.sync.dma_start(x8_t[:, :, 256:768], x8_d.ap()[:, :, 256:768])
        nc.sync.dma_start(xl_t[:, :, 256:768], xl_d.ap()[:, :, 256:768])
        nc.sync.dma_start(x8_t[:, :, 0:256], x8_d.ap()[:, :, 0:256])
        nc.sync.dma_start(xl_t[:, :, 0:256], xl_d.ap()[:, :, 0:256])

        # warm the exp table off the critical path
        warm_t = st.tile([1, 2], F32, name="warm")
        nc.scalar.activation(warm_t[:], cf_t[0:1, 0:2], EXP,
                             bias=0.0, scale=SCALE)

        # ---- P~ / V~ tiles (SBUF-resident until the duo's A@V) ----
        p_pair = {}   # (duo, hh, p) -> [128, 2, Wp] fp8
        p_tail = {}   # (duo, hh, kt in 14,15) -> [128, W] fp16
        vp_pair = {}  # (duo, p) -> [128, 2, 128] fp8   (both heads in free)
        vp_tail = {}  # (duo, kt) -> [128, 128] fp16
        for d in range(2):
            for hh in range(2):
                for p in range(NPAIR):
                    wp = T - 256 * p
                    p_pair[(d, hh, p)] = pp.tile(
                        [128, 2, wp], F8, tag=f"p{d}_{hh}_{p}",
                        name=f"p{d}_{hh}_{p}")
                for kt in (14, 15):
                    p_tail[(d, hh, kt)] = pp.tile(
                        [128, T - 128 * kt], F16, tag=f"pt{d}_{hh}_{kt}",
                        name=f"pt{d}_{hh}_{kt}")
            for kt in (14, 15):
                vp_tail[(d, kt)] = vp.tile(
                    [128, 128], F16, tag=f"vpt{d}_{kt}",
                    name=f"vpt{d}_{kt}")
            for p in range(NPAIR):
                vp_pair[(d, p)] = vp.tile(
                    [128, 2, 128], F8, tag=f"vp{d}_{p}",
                    name=f"vp{d}_{p}")
        # zero the odd-kt first-128 strips (masked region the exp never
        # writes); gpsimd keeps this off the busy engines
        for d in range(2):
            for hh in range(2):
                for p in range(NPAIR):
                    nc.gpsimd.memset(p_pair[(d, hh, p)][:, 1, 0:128], 0.0)

        # ---- PE helper emitters ----
        pv_tog = [0]

        def pv_tile(name):
            tag = ("pj", "ob")[pv_tog[0] % 2]
            pv_tog[0] += 1
            return pv_pool.tile([128, 512], F32, tag=tag, name=name)

        def qk_src(c0, n):
            # (x8 slice fn, xl slice fn) for global query cols [c0, c0+n)
            assert c0 + n <= T
            if c0 >= 1920:
                lo = c0 - 1920
                return (lambda ep: x8h_t[:, 2 * ep:2 * ep + 2, lo:lo + n],
                        lambda ep: xlh_t[:, 2 * ep:2 * ep + 2, lo:lo + n])
            if c0 >= XMAIN:
                lo = c0 - XMAIN
                assert c0 + n <= 1920
                return (lambda ep: x8b_t[:, 2 * ep:2 * ep + 2, lo:lo + n],
                        lambda ep: xlb_t[:, 2 * ep:2 * ep + 2, lo:lo + n])
            assert c0 + n <= XMAIN
            return (lambda ep: x8_t[:, 2 * ep:2 * ep + 2, c0:c0 + n],
                    lambda ep: xl_t[:, 2 * ep:2 * ep + 2, c0:c0 + n])

        _qkw = {(d, k): set() for d in range(2) for k in (False, True)}

        def emit_qk_chunk(duo, is_k, c0, n=512):
            _qkw[(duo, is_k)].update(range(c0, c0 + n))
            # one chunk of the Q^T/K^T projection for `duo`, global query
            # cols [c0, c0+n): psum = X8.T@W8 + Xl.T@W8 + X8.T@Wl
            # (12 fp8 DoubleRow matmuls) + bias -> bf16 SBUF.
            for cut in (XMAIN, 1920):
                if c0 < cut < c0 + n:
                    emit_qk_chunk(duo, is_k, c0, cut - c0)
                    emit_qk_chunk(duo, is_k, cut, c0 + n - cut)
                    return
            w8o = duo * 512 + (128 if is_k else 0)
            wlo = w8o + 256
            out_t, b_t = (kt_t, bkc_t) if is_k else (qt_t, bqc_t)
            s8, sl = qk_src(c0, n)
            ps = pv_tile(f"qk{duo}_{int(is_k)}_{c0}")
            first = True
            for pidx, (xs, wo) in enumerate(
                    ((s8, w8o), (s8, wlo), (sl, w8o))):
                for ep in range(4):
                    nc.tensor.matmul(
                        ps[:, 0:n],
                        lhsT=w_t[:, 2 * ep:2 * ep + 2, wo:wo + 128],
                        rhs=xs(ep),
                        start=first,
                        stop=(pidx == 2 and ep == 3),
                        perf_mode=DRM,
                    )
                    first = False
            nc.vector.tensor_scalar_add(
                out_t[:, duo, c0:c0 + n], ps[:, 0:n],
                b_t[:, duo:duo + 1])

        _vw = set()        # v_t tiles written
        _vtw = {0: set(), 1: set()}   # vtilde done per duo
        _expw = {0: set(), 1: set()}  # (kt,hh) exps done per duo

        def emit_v_tile(kt):
            _vw.add(kt)
            # V tile (both duos): [128 t, 256 d] = X.T@Wv*32 + 32*bv
            s8, sl = qk_src(kt * 128, 128)
            ps = pv_tile(f"v{kt}")
            pvs = ps[:, 0:D2]
            for si, (xs, wo) in enumerate(((s8, V8O), (sl, V8O), (s8, VLO))):
                for ep in range(4):
                    nc.tensor.matmul(
                        pvs,
                        lhsT=xs(ep),
                        rhs=w_t[:, 2 * ep:2 * ep + 2, wo:wo + D2],
                        start=(si == 0 and ep == 0),
                        stop=False,
                        perf_mode=DRM,
                    )
            nc.tensor.matmul(pvs, lhsT=ones_t[0:1, 0:128], rhs=bvr_t[:],
                             start=False, stop=True)
            nc.vector.tensor_copy(v_t[:, kt, :], pvs)

        def emit_scores_exp(duo, kt, fillers=()):
            # scores S^T[key, q] for q in [qlo, T), exp'd into P~ with
            # per-key bias -ln(c_k); accum -> rs (per-key sums r~).
            # `fillers`: PE work emitted between score/exp units so the
            # engine pipeline never leaves ACT waiting on the next scores.
            fillers = list(fillers)
            qlo = kt * 128
            w = T - qlo
            need_q = set(range(qlo, T))
            need_k = set(range(qlo, qlo + 128))
            mq = need_q - _qkw[(duo, False)]
            mk = need_k - _qkw[(duo, True)]
            assert not mq, f"scores({duo},{kt}) missing Q cols {sorted(mq)[:4]}..{sorted(mq)[-1]}"
            assert not mk, f"scores({duo},{kt}) missing K cols {sorted(mk)[:4]}..{sorted(mk)[-1]}"
            pieces = [(0, min(w, 1536))]
            if w > 1536:
                pieces.append((1536, w - 1536))
            rs_t = st.tile([128, 2], F32, tag="rs", name=f"rs{duo}_{kt}",
                           bufs=20)
            sums_t = (st.tile([128, 4], F32, tag="sums", name=f"sm{duo}_{kt}")
                      if len(pieces) > 1 else None)
            # pieces OUTER, heads inner; piece-1 (1536 wide) ping-pongs
            # the two dedicated score slots while the narrow piece-2
            # (<=512) borrows a proj/V psum slot (separate bank, its own
            # singleton matmul groups) -- so a score slot is reused only
            # after the other head's full-width exp plus both piece-2
            # exps have drained.
            for pi, (poff, pw) in enumerate(pieces):
                for hh in range(2):
                    d0 = 64 * hh
                    if fillers and (pi, hh) in ((0, 1), (1, 0)):
                        fillers.pop(0)()
                    if poff == 0:
                        sc = sc_pool.tile([128, 1536], F32, tag="sc",
                                          name="sc")
                    else:
                        sc = pv_tile("p2sc")
                    for co in range(0, pw, 512):
                        n = min(512, pw - co)
                        nc.tensor.matmul(
                            sc[:, co:co + n],
                            lhsT=kt_t[d0:d0 + 64, duo, qlo:qlo + 128],
                            rhs=qt_t[d0:d0 + 64, duo,
                                     qlo + poff + co:qlo + poff + co + n],
                            start=True,
                            stop=not (poff == 0 and co == 0),
                        )
                    if poff == 0:
                        nc.tensor.matmul(
                            sc[:, 0:128], lhsT=masku_t[:, 0:128],
                            rhs=ident_t[:], start=False, stop=True,
                            skip_group_check=True)
                    if kt >= 14:
                        dst = p_tail[(duo, hh, kt)][:, poff:poff + pw]
                    else:
                        p = kt // 2
                        par = kt % 2
                        off = 128 * par + poff
                        dst = p_pair[(duo, hh, p)][:, par, off:off + pw]
                    acc = (sums_t[:, hh * 2 + pi:hh * 2 + pi + 1]
                           if sums_t is not None else rs_t[:, hh:hh + 1])
                    nc.scalar.activation(
                        dst, sc[:, 0:pw], EXP,
                        bias=biasc_t[:, kt:kt + 1], scale=SCALE,
                        accum_out=acc)
            _expw[duo].add((kt, 0)); _expw[duo].add((kt, 1))
            if sums_t is not None:
                for hh in range(2):
                    nc.vector.reduce_sum(
                        rs_t[:, hh:hh + 1], sums_t[:, hh * 2:hh * 2 + 2],
                        axis=AX)
            for f in fillers:
                f()
            return rs_t

        def emit_vtilde(duo, kt, rs_t, eng=None):
            assert kt in _vw, f"vtilde({duo},{kt}) before v_tile"
            assert (kt, 0) in _expw[duo] and (kt, 1) in _expw[duo], \
                f"vtilde({duo},{kt}) before exps"
            _vtw[duo].add(kt)
            # rinv = 1/r~ ; V~ = 32*v*rinv (fp8 pairs / fp16 tail).
            # The SBUF->SBUF multiplies run on the otherwise-idle gpsimd
            # engine to keep DVE free for PSUM drains (gpsimd has no PSUM
            # port, so only this step can move there).
            rinv_t = st.tile([128, 2], F32, tag="rinv", name=f"ri{duo}_{kt}")
            nc.vector.reciprocal(rinv_t[:], rs_t[:])
            eng = eng or nc.vector
            for hh in range(2):
                if kt >= 14:
                    dst = vp_tail[(duo, kt)][:, 64 * hh:64 * hh + 64]
                else:
                    dst = vp_pair[(duo, kt // 2)][:, kt % 2,
                                                  64 * hh:64 * hh + 64]
                eng.tensor_scalar_mul(
                    dst,
                    v_t[:, kt, duo * 128 + 64 * hh:duo * 128 + 64 * hh + 64],
                    rinv_t[:, hh:hh + 1])

        av_psum = {}

        def emit_av_qtile(duo, j, ot_sb, part="all"):
            for kt in range(min(j, 13) + 1 if part == "pairs" else j + 1):
                if part != "tail" or kt >= 14:
                    assert kt in _vtw[duo], \
                        f"av({duo},{j},{part}) before vtilde({duo},{kt})"
                    for hh in range(2):
                        assert (kt, hh) in _expw[duo], \
                            f"av({duo},{j},{part}) before exp({duo},{kt},{hh})"

            # O[q, d] for q-tile j: fp8 DoubleRow, P~ stationary, V~ moving
            # with both heads side-by-side; fp16 solo for key-tiles 14, 15.
            # Out partitions = queries (always base 0).
            # part="pairs" emits only the fp8 pair matmuls (group left
            # open); part="tail" finishes kt14/15 + scale + DMA -- used to
            # shorten the dependency chain after the final exp.
            ob = pv_tile(f"av{duo}_{j}")
            obq = ob[:, 0:128]
            plast = min(j // 2, NPAIR - 1)
            # pairs AND fp16 tails together per head: the accumulation
            # group on each 64-col psum slice must open and close with no
            # other group starting on the same bank in between.
            for hh in range(2):
                for p in range(plast + 1):
                    c0 = 128 * j - 256 * p
                    nc.tensor.matmul(
                        obq[:, 64 * hh:64 * hh + 64],
                        lhsT=p_pair[(duo, hh, p)][:, :, c0:c0 + 128],
                        rhs=vp_pair[(duo, p)][:, :, 64 * hh:64 * hh + 64],
                        start=(p == 0),
                        stop=(j < 14 and p == plast),
                        perf_mode=DRM,
                        skip_group_check=True,
                    )
                for kt in (14, 15):
                    if kt > j:
                        continue
                    c0 = 128 * j - 128 * kt
                    nc.tensor.matmul(
                        obq[:, 64 * hh:64 * hh + 64],
                        lhsT=p_tail[(duo, hh, kt)][:, c0:c0 + 128],
                        rhs=vp_tail[(duo, kt)][:, 64 * hh:64 * hh + 64],
                        start=False,
                        stop=(kt == min(j, 15)),
                        skip_group_check=True,
                    )
            nc.vector.tensor_scalar_mul(
                ot_sb[:, 128 * j:128 * j + 128], obq, 1.0 / 32.0)
            nc.sync.dma_start(
                ot_d.ap()[128 * j:128 * j + 128,
                          duo * 128:duo * 128 + 128],
                ot_sb[:, 128 * j:128 * j + 128])

        # PE p-state warmup: the cost model runs the PE at 0.65/1.2 GHz
        # until it has been continuously busy for ~3us.  Burn that ramp on
        # dummy matmuls over a zeroed strip (no DMA dependency) so the
        # first real projection chunks run at full clock the moment their
        # DMA lands.  Results land in a rotating proj psum slot that the
        # first real chunk simply overwrites.
        # PE p-state warmup: the cost model runs the PE at 0.65/1.2 GHz
        # until it has been continuously busy for ~3us.  Burn the ramp on
        # dummy matmuls over a zeroed strip (no DMA dependency) so the
        # first real projection chunks run at full clock when their DMA
        # lands; sized to end just after the wqk0/cst8h transfers.
        wm = p_pair[(0, 0, 0)][:, 1, 0:128]   # zeroed strip
        for i in range(0, 36, 2):
            wps = pv_tile(f"wm{i}")
            for k in range(2):
                nc.tensor.matmul(
                    wps[:, 0:128], lhsT=wm, rhs=wm,
                    start=True, stop=True, skip_group_check=True)

        # ---- schedule ----
        # duo0: key-tiles DESCENDING (small score tiles first -> early exp
        # start under partial DMA); duo1: ASCENDING so its own A@V q-tiles
        # interleave as soon as their key-pairs complete (short tail).
        #
        # Fillers (proj chunks / V tiles / A@V) run on PE/DVE between
        # score-exp subunits.  Each unit's filler PE-time is kept under
        # its own exp window, chunks are 256 wide, and every chunk sits at
        # its LATEST deadline (the unit just before its columns are first
        # read) so the early small-exp units stay lean.
        rs_pend = {}

        def CH(duo, is_k, c0):
            return lambda: emit_qk_chunk(duo, is_k, c0, 256)

        def VF(jj):
            def f():
                emit_v_tile(jj)
                emit_vtilde(0, jj, rs_pend.pop(jj))
            return f

        # duo0 phase carries ONLY projection chunks (V~ fills move into
        # duo1's phase, which has PE slack); kt0's unit stays empty so
        # duo1 kt0's scores follow duo0 kt0's with no leftover-filler
        # delay at the seam.
        def CHn(duo, is_k, c0, n):
            return lambda: emit_qk_chunk(duo, is_k, c0, n)

        d0_fill = {
            15: [CHn(0, False, 1792, 128), CHn(0, True, 1792, 128)],
            14: [CHn(0, False, 1664, 128), CHn(0, True, 1664, 128)],
            13: [CHn(0, False, 1536, 128), CHn(0, True, 1536, 128)],
            12: [CH(0, False, 1280), CH(0, True, 1280)],
            11: [CH(0, False, 1024), CH(0, True, 1024)],
            10: [VF(15)],
            9: [VF(14), VF(13)],
            8: [CH(0, False, 768), CH(0, True, 768)],
            7: [VF(12), VF(11)],
            6: [CH(0, False, 512), CH(0, True, 512)],
            5: [VF(10), VF(9)],
            4: [CH(0, False, 256), CH(0, True, 256),
                CH(1, False, 1792), CH(1, False, 1536)],
            3: [CH(1, False, 1280), CH(1, False, 1024), VF(8)],
            2: [CH(0, False, 0), CH(0, True, 0), CH(1, False, 768),
                VF(7)],
            1: [CH(1, False, 512), CH(1, False, 256), VF(6), VF(5)],
            0: [CH(1, False, 0), CH(1, True, 0), VF(4)],
        }

        # head: only the [1920:2048] q/k chunks (from the small cst8h
        # DMA) run before kt15's scores -- everything else fills in
        # behind the exp stream
        emit_qk_chunk(0, False, 1920, n=128)
        emit_qk_chunk(0, True, 1920, n=128)

        ot_sbs = [osb.tile([128, T], F32, tag=f"osb{d}", name=f"osb{d}")
                  for d in range(2)]
        # duo0's 16 A@V q-tiles, injected into duo1's phase (shifted two
        # units vs the V~ fills they consume)
        av0_sched = {2: (0, 1), 3: (2, 3), 4: (4, 5), 5: (6, 7),
                     6: (8, 9), 7: (10, 11), 8: (12, 13), 9: (14, 15)}
        # duo1 K chunks + the trailing duo0 V~ fills, spread across
        # duo1's units ahead of their consumers (vtilde(1,j) at unit j+1
        # and av0 pairs per av0_sched)
        d1_fill = {0: [VF(0), VF(1), CH(1, True, 256)],
                   1: [VF(2), VF(3), CH(1, True, 512)],
                   2: [CH(1, True, 768)],
                   5: [CH(1, True, 1024)], 7: [CH(1, True, 1280)],
                   9: [CH(1, True, 1536)], 11: [CH(1, True, 1792)]}

        for kt in range(NKT - 1, -1, -1):
            rs_pend[kt] = emit_scores_exp(0, kt, d0_fill.get(kt, ()))
        rs_prev = None
        for kt in range(NKT):
            fills = []
            if kt >= 1:
                pk = kt - 1
                fills.append(
                    lambda p=pk, r=rs_prev: emit_vtilde(1, p, r))
                if pk % 2 == 1 and pk <= 13:
                    fills.append(
                        lambda p=pk: emit_av_qtile(1, p - 1, ot_sbs[1]))
                    fills.append(
                        lambda p=pk: emit_av_qtile(1, p, ot_sbs[1]))
                elif pk == 14:
                    fills.append(
                        lambda: emit_av_qtile(1, 14, ot_sbs[1]))
            for j in av0_sched.get(kt, ()):
                fills.append(lambda jj=j: emit_av_qtile(0, jj, ot_sbs[0]))
            fills.extend(d1_fill.get(kt, ()))
            rs_prev = emit_scores_exp(1, kt, fills)
        emit_vtilde(1, 15, rs_prev, eng=nc.vector)
        emit_av_qtile(1, 15, ot_sbs[1])


def _get_module():
    if "nc" not in _CACHE:
        _CACHE["nc"] = _build_module()
    return _CACHE["nc"]


def _host_tables():
    k = np.arange(T)
    c = np.where(
        k < T - 256,
        2.0 ** np.round(0.5 * np.log2(1.031 * (T - k))),
        1.0)
    biasc = (-np.log(c)).reshape(NKT, 128).T.astype(np.float32)
    qi = np.arange(128)
    masku = np.where(qi[:, None] < qi[None, :], NEG, 0.0).astype(np_bf16)
    ident = np.eye(128, dtype=np.float32).astype(np_bf16)
    ones = np.ones((128, 128), np.float32).astype(np_bf16)
    return biasc, masku, ident, ones


def _split8(a):
    hi = a.astype(np_f8)
    lo = (a - hi.astype(np.float32)).astype(np_f8)
    return hi, lo


def _make_in_maps(X, Wq, bq, Wk, bk, Wv, bv):
    X = np.asarray(X, np.float32)
    biasc, masku, ident, ones = _host_tables()
    in_maps = []
    for core in range(8):
        b, g = divmod(core, 4)
        rows = slice(D2 * g, D2 * g + D2)
        xt = np.ascontiguousarray(X[b].T)              # [E, T]
        x8, xl = _split8(xt)
        x8 = np.ascontiguousarray(x8.reshape(8, 128, T).transpose(1, 0, 2))
        xl = np.ascontiguousarray(xl.reshape(8, 128, T).transpose(1, 0, 2))

        def wprep(Wfull):
            ws = np.asarray(Wfull)[rows].T.astype(np.float32) * WS  # [E, 256]
            return _split8(ws)

        wq8, wql = wprep(Wq)
        wk8, wkl = wprep(Wk)
        wv8, wvl = wprep(Wv)
        wqk = np.concatenate(
            [wq8[:, :128], wk8[:, :128], wql[:, :128], wkl[:, :128],
             wq8[:, 128:], wk8[:, 128:], wql[:, 128:], wkl[:, 128:]],
            axis=1)  # [E, 1024], duo-major
        wv8l = np.concatenate([wv8, wvl], axis=1)

        def dr3(w):  # [E, n] -> [128, 8, n]
            n = w.shape[1]
            return np.ascontiguousarray(
                w.reshape(8, 128, n).transpose(1, 0, 2))

        bqc = (WS * np.asarray(bq)[rows]).reshape(2, 128).T
        bkc = (WS * np.asarray(bk)[rows]).reshape(2, 128).T
        cf = np.concatenate([biasc, bqc, bkc], axis=1).astype(np.float32)

        # pack cst8h: planes 0-7 x8[1920:2048], 8-15 xl[...], 16 cf
        # (raw bytes viewed as fp8)
        cst8h = np.zeros((128, 17, 128), np.uint8)
        cst8h[:, 0:8, :] = x8[:, :, 1920:T].view(np.uint8)
        cst8h[:, 8:16, :] = xl[:, :, 1920:T].view(np.uint8)
        cst8h[:, 16, 0:80] = np.ascontiguousarray(cf).view(np.uint8)
        bvrp = np.zeros((128, 256), np_bf16)
        bvrp[0] = (WS * np.asarray(bv)[rows]).astype(np_bf16)
        mi = np.ascontiguousarray(
            np.concatenate([masku, ident, ones, bvrp], axis=1))

        cst8b = np.zeros((128, 16, 128), np.uint8)
        cst8b[:, 0:8, :] = x8[:, :, XMAIN:1920].view(np.uint8)
        cst8b[:, 8:16, :] = xl[:, :, XMAIN:1920].view(np.uint8)

        in_maps.append({
            "cst8h": cst8h.view(np_f8),
            "cst8b": cst8b.view(np_f8),
            "mi": mi,
            "x8": np.ascontiguousarray(x8[:, :, 0:XMAIN]),
            "xl": np.ascontiguousarray(xl[:, :, 0:XMAIN]),
            "wqk": dr3(wqk), "wv8l": dr3(wv8l),
        })
    return in_maps


def kernel(X, Wq, bq, Wk, bk, Wv, bv, **kw):
    in_maps = _make_in_maps(X, Wq, bq, Wk, bk, Wv, bv)
    nc = _get_module()
    res = run_bass_kernel_spmd(nc, in_maps, core_ids=list(range(8)), **kw)
    _CACHE["last_res"] = res
    out = np.zeros((B, T, E), np.float32)
    for c in range(8):
        b, g = divmod(c, 4)
        out[b, :, D2 * g:D2 * g + D2] = res.results[c]["ot"]
    return out


if __name__ == "__main__":
    _get_module()
    print("module built ok")

